# revision 36
# baseline (speedup 1.0000x reference)
"""Neighbourhood attention block (7x7 clamped window) on 8 Trainium2 cores.

Sharding: (batch, head-pair) tensor parallel. Core c handles batch b = c//4
and heads (2*(c%4), 2*(c%4)+1). Each core computes q/k/v projections for its
two heads, neighbourhood attention, and a partial output projection
y_partial = ao @ w_out_slice^T (bf16). Host sums the 4 partials per batch.

v2 layout (vs baseline): everything bf16 off-host (halves DMA + enables DVE
fast modes), input DMA chunked so projections start early, scores for the two
heads paired into one PSUM tile (one exp + one mask-mul per chunk-tile pair),
denominator rides the PV staging copy, normalize done per 512-token chunk
(reciprocal + partition_broadcast + bf16 mul) feeding the output projection
pipeline.

Attention layout: scoresT [key, query] tiles so PV needs no transposes.
Keys are chunked 2 image rows (128 tokens) per chunk; each chunk is matched
against the 8 query rows that can see it (512 queries). Masking is
multiplicative 0/1 after exp (bf16), so invalid keys contribute 0 to both the
PV numerator and the denominator (a ones-column appended to V).
"""
import os
import numpy as np
import ml_dtypes
from contextlib import ExitStack

_PHASES = os.environ.get("KERNEL_PHASES", "1234")  # debug bisect knob

import concourse.bass as bass
import concourse.bacc as bacc
import concourse.tile as tile
import concourse.mybir as mybir
from concourse.bass_utils import run_bass_kernel_spmd
from concourse.masks import make_identity

F32 = mybir.dt.float32
BF16 = mybir.dt.bfloat16

B, H, W, D = 2, 64, 64, 512
DH, NH = 64, 8
S = H * W              # 4096 tokens per batch
KER = 7
SCALE = DH ** -0.5     # 0.125
NCORES = 8

# ---------------------------------------------------------------- geometry

def _sh(r):            # clamped window start (rows); same formula for cols
    return min(max(r - KER // 2, 0), H - KER)


def _chunks_of_row(r):  # key chunks (2 rows each) seen by query row r
    s = _sh(r)
    return list(range(s // 2, (s + KER + 1) // 2))


def _build_plan():
    """Tiles: scoresT [128 keys of chunk c, qw queries at q0]. Groups: PV
    accumulations [65, qw] covering disjoint query ranges."""
    tiles = []          # dict(c, q0, qw)
    for c in range(32):
        q0r = min(max(2 * c - 3, 0), 56)
        tiles.append(dict(c=c, q0=q0r * 64, qw=512))
    for c in (2, 3):        # query rows 0..2 miss these chunks' main windows
        tiles.append(dict(c=c, q0=0, qw=192))
    for c in (28, 29):      # query rows 61..63
        tiles.append(dict(c=c, q0=61 * 64, qw=192))

    # rows covered by each tile, for resolution
    def covers(t, r):
        return t["q0"] <= r * 64 and (r + 1) * 64 <= t["q0"] + t["qw"]

    groups = []         # dict(q0, qw, rows)
    groups.append(dict(rows=[0]))
    groups.append(dict(rows=[1, 2]))
    for k in range(14):
        r0 = 4 * k + 3
        groups.append(dict(rows=[r0, r0 + 1, r0 + 2, r0 + 3]))
    groups.append(dict(rows=[59, 60, 61, 62]))
    groups.append(dict(rows=[63]))

    for g in groups:
        rows = g["rows"]
        g["q0"] = rows[0] * 64
        g["qw"] = len(rows) * 64
        # chunk -> contiguous row subrange of this group needing it
        chunk_rows = {}
        for r in rows:
            for c in _chunks_of_row(r):
                a, b = chunk_rows.get(c, (r, r))
                chunk_rows[c] = (min(a, r), max(b, r))
        mms = []        # (c, row_a, row_b_inclusive, width)
        for c, (ra, rb) in sorted(chunk_rows.items()):
            mms.append((c, ra, rb, (rb - ra + 1) * 64))
        mms.sort(key=lambda m: -m[3])   # widest (full-group) first for start=True
        assert mms[0][3] == g["qw"], (g, mms)
        # resolve each (c, row range) to probs-tile segments
        segs = []       # (c, tile_i, tile_off, out_off, length)
        for c, ra, rb, _w in mms:
            r = ra
            while r <= rb:
                cand = [i for i, t in enumerate(tiles) if t["c"] == c and covers(t, r)]
                assert cand, (g, c, r)
                ti = cand[0]
                t = tiles[ti]
                # extend run while same tile covers
                r2 = r
                while r2 + 1 <= rb and covers(t, r2 + 1):
                    r2 += 1
                segs.append((c, ti, r * 64 - t["q0"], r * 64 - g["q0"],
                             (r2 - r + 1) * 64))
                r = r2 + 1
        g["segs"] = segs

    # sanity: every (query row, chunk) incidence consumed exactly once
    seen = set()
    for g in groups:
        for c, ti, toff, ooff, ln in g["segs"]:
            for r in range((g["q0"] + ooff) // 64, (g["q0"] + ooff + ln) // 64):
                key = (r, c)
                assert key not in seen, key
                seen.add(key)
    for r in range(H):
        for c in _chunks_of_row(r):
            assert (r, c) in seen, (r, c)

    # masks per tile (0/1), deduped
    starts = np.minimum(np.maximum(np.arange(H) - KER // 2, 0), H - KER)
    valid = (np.arange(H)[None, :] >= starts[:, None]) & \
            (np.arange(H)[None, :] < starts[:, None] + KER)   # [q pos, k pos]

    def tile_mask(t):
        ktok = t["c"] * 128 + np.arange(128)
        qtok = t["q0"] + np.arange(t["qw"])
        m = np.zeros((128, 512), np.float32)
        m[:, :t["qw"]] = (valid[qtok[None, :] // 64, ktok[:, None] // 64]
                          & valid[qtok[None, :] % 64, ktok[:, None] % 64])
        return m

    mask_list, mask_ids = [], {}
    for t in tiles:
        m = tile_mask(t)
        key = m.tobytes()
        if key not in mask_ids:
            mask_ids[key] = len(mask_list)
            mask_list.append(m)
        t["mask_id"] = mask_ids[key]
    return tiles, groups, np.stack(mask_list)


TILES, GROUPS, MASKS = _build_plan()
NMASK = len(MASKS)

# ---------------------------------------------------------------- device

_NC_CACHE = {}
TRACE = False          # set True (e.g. from test.py) to capture an NTFF profile
LAST_RESULTS = None    # BassKernelResults of the most recent kernel() call


def _build_module():
    nc = bacc.Bacc("TRN2", target_bir_lowering=False, debug=False,
                   num_devices=NCORES)
    xT_d = nc.dram_tensor("xT", [D, S], BF16, kind="ExternalInput")
    wq_d = nc.dram_tensor("wq", [D, 128], BF16, kind="ExternalInput")
    wk_d = nc.dram_tensor("wk", [D, 128], BF16, kind="ExternalInput")
    wv_d = nc.dram_tensor("wv", [D, 128], BF16, kind="ExternalInput")
    wo_d = nc.dram_tensor("wo", [128, 512], BF16, kind="ExternalInput")
    mk_d = nc.dram_tensor("masks", [NMASK, 128, 2, 512], BF16,
                          kind="ExternalInput")
    y_d = nc.dram_tensor("y", [S, D], BF16, kind="ExternalOutput")

    NB = 8              # 512-token projection chunks

    with tile.TileContext(nc) as tc, ExitStack() as ctx:
        const = ctx.enter_context(tc.tile_pool(name="const", bufs=1))
        wq_t = const.tile([128, 4, 128], BF16, tag="wq")
        nc.sync.dma_start(out=wq_t[:], in_=wq_d.ap().rearrange("(c p) m -> p c m", p=128))
        wk_t = const.tile([128, 4, 128], BF16, tag="wk")
        nc.sync.dma_start(out=wk_t[:], in_=wk_d.ap().rearrange("(c p) m -> p c m", p=128))
        wv_t = const.tile([128, 4, 128], BF16, tag="wv")
        nc.sync.dma_start(out=wv_t[:], in_=wv_d.ap().rearrange("(c p) m -> p c m", p=128))

        # x arrives in NB token chunks so projections can start early;
        # wo + masks queue after x (not needed until attention starts)
        xT_t = const.tile([128, 4, S], BF16, tag="xT")
        for nb in range(NB):
            sl = slice(nb * (S // NB), (nb + 1) * (S // NB))
            nc.sync.dma_start(
                out=xT_t[:, :, sl],
                in_=xT_d.ap()[:, sl].rearrange("(c p) t -> p c t", p=128))
        wo_t = const.tile([128, 512], BF16, tag="wo")
        nc.sync.dma_start(out=wo_t[:], in_=wo_d[:, :])
        mk_t = const.tile([128, NMASK, 2, 512], BF16, tag="mk")
        nc.sync.dma_start(out=mk_t[:], in_=mk_d.ap().rearrange("m p c w -> p m c w"))

        qT = const.tile([128, S], BF16, tag="qT")      # [2 heads x 64e, tok]
        kT = const.tile([128, S], BF16, tag="kT")
        vT = const.tile([128, S], BF16, tag="vT")
        # V: [tok_in_chunk, chunk, 130]: cols 0:64 u0-e, 64 ones, 65:129 u1-e, 129 ones
        V = const.tile([128, 32, 130], BF16, tag="V")
        nc.gpsimd.memset(V[:, :, 64:65], 1.0)   # only the ones-columns; the
        nc.gpsimd.memset(V[:, :, 129:130], 1.0)  # rest is overwritten below
        st = const.tile([65, 2, S], BF16, tag="st")    # unnormalized pv + denom row
        rec = const.tile([1, 2, S], BF16, tag="rec")   # 1/denom per unit
        rb0 = const.tile([64, S], BF16, tag="rb0")     # recip broadcast per unit
        rb1 = const.tile([64, S], BF16, tag="rb1")
        aoT = const.tile([128, S], BF16, tag="aoT")
        ident = const.tile([128, 128], BF16, tag="ident")
        make_identity(nc, ident[:])

        def copy_on(eng, dst, src):
            if eng is nc.scalar:
                eng.activation(dst, src, mybir.ActivationFunctionType.Copy)
            else:
                eng.tensor_copy(dst, src)

        cp_i = 0

        def copy(dst, src, engs=(nc.vector, nc.scalar, nc.gpsimd)):
            nonlocal cp_i
            eng = engs[cp_i % len(engs)]
            cp_i += 1
            copy_on(eng, dst, src)

        # ---- phase 1: projections
        CB = S // NB
        with tc.tile_pool(name="pps", bufs=3, space="PSUM") as pps:
            for nb in range(NB) if "1" in _PHASES else ():
                sl = slice(nb * CB, (nb + 1) * CB)
                for w_t, dst in ((wq_t, qT), (wk_t, kT), (wv_t, vT)):
                    acc = pps.tile([128, 512], F32, tag="acc")
                    for dc in range(4):
                        nc.tensor.matmul(acc[:], w_t[:, dc, :],
                                         xT_t[:, dc, sl],
                                         start=(dc == 0), stop=(dc == 3))
                    copy(dst[:, sl], acc[:],
                         engs=(nc.vector, nc.vector, nc.scalar))
                # V transpose for the 4 chunks of this nb (vT cols now ready)
                for pci in range(nb * 2, nb * 2 + 2):
                    tp = pps.tile([128, 2, 128], BF16, tag="tp")
                    for s in range(2):
                        ci = pci * 2 + s
                        nc.tensor.transpose(tp[:, s, :],
                                            vT[:, ci * 128:(ci + 1) * 128],
                                            ident[:])
                    # gpsimd cannot read PSUM: these stay on DVE/Act
                    copy(V[:, pci * 2:pci * 2 + 2, 0:64], tp[:, :, 0:64],
                         engs=(nc.vector, nc.scalar))
                    copy(V[:, pci * 2:pci * 2 + 2, 65:129], tp[:, :, 64:128],
                         engs=(nc.vector, nc.scalar))

        # ---- phase 2+3+4: attention, normalize, output projection
        # normalize chunk cn (512 tokens = 8 query rows) is emitted as soon as
        # the groups covering those rows are done; output projection follows.
        with tc.tile_pool(name="scp", bufs=2, space="PSUM") as scp, \
             tc.tile_pool(name="pvp", bufs=2, space="PSUM") as pvp, \
             tc.tile_pool(name="opp", bufs=2, space="PSUM") as opp, \
             tc.tile_pool(name="prp", bufs=10) as prp, \
             tc.tile_pool(name="yvp", bufs=4) as yvp:
            emitted = {}

            def emit_pair(ti):
                t = TILES[ti]
                qw, c = t["qw"], t["c"]
                sc = scp.tile([128, 2, 512], F32, tag="sc")
                for u in (0, 1):
                    ue = slice(u * 64, u * 64 + 64)
                    nc.tensor.matmul(sc[:, u, :qw],
                                     kT[ue, c * 128:(c + 1) * 128],
                                     qT[ue, t["q0"]:t["q0"] + qw],
                                     start=True, stop=True)
                pr = prp.tile([128, 2, 512], BF16, tag="pr")
                nc.scalar.activation(pr[:, :, :qw], sc[:, :, :qw],
                                     mybir.ActivationFunctionType.Exp,
                                     scale=SCALE)
                if qw == 512:   # flat 2D APs qualify for DVE fast modes
                    nc.vector.tensor_mul(
                        pr[:].rearrange("p c w -> p (c w)"),
                        pr[:].rearrange("p c w -> p (c w)"),
                        mk_t[:, t["mask_id"]].rearrange("p c w -> p (c w)"))
                else:
                    nc.vector.tensor_mul(pr[:, :, :qw], pr[:, :, :qw],
                                         mk_t[:, t["mask_id"], :, :qw])
                emitted[ti] = pr

            def do_group(g):
                qw = g["qw"]
                sl = slice(g["q0"], g["q0"] + qw)
                nseg = len(g["segs"])
                pv = pvp.tile([65, 2, 256], F32, tag="pv")   # both units, 1 bank
                for u in (0, 1):
                    uv = slice(u * 65, u * 65 + 65)
                    for si, (c, ti, toff, ooff, ln) in enumerate(g["segs"]):
                        nc.tensor.matmul(pv[:, u, ooff:ooff + ln],
                                         V[:, c, uv],
                                         emitted[ti][:, u, toff:toff + ln],
                                         start=(si == 0), stop=(si == nseg - 1))
                copy_on(nc.vector, st[:, 0, sl], pv[:, 0, :qw])
                copy_on(nc.scalar, st[:, 1, sl], pv[:, 1, :qw])
                with nc.allow_low_precision(reason="denominators are O(1)-O(1e3); bf16 recip ~0.4% rel err, gate is 2e-2"):
                    nc.vector.reciprocal(rec[:, :, sl], st[64:65, :, sl])

            def normalize(g):
                sl = slice(g["q0"], g["q0"] + g["qw"])
                for u, rb in ((0, rb0), (1, rb1)):
                    ud = slice(u * 64, u * 64 + 64)
                    nc.gpsimd.partition_broadcast(rb[:, sl], rec[0:1, u, sl])
                    # all-SBUF bf16 mul: legal on gpsimd; alternate to spread load
                    eng = nc.gpsimd if u else nc.vector
                    eng.tensor_mul(aoT[ud, sl], st[0:64, u, sl], rb[:, sl])

            def project(tcn):     # one 128-token chunk: matmul, stage, DMA out
                acc = opp.tile([128, 512], F32, tag="oacc")
                nc.tensor.matmul(acc[:], aoT[:, tcn * 128:(tcn + 1) * 128],
                                 wo_t[:], start=True, stop=True)
                yv = yvp.tile([128, 512], BF16, tag="yv")
                copy(yv[:], acc[:], engs=(nc.scalar, nc.vector))
                nc.sync.dma_start(out=y_d[tcn * 128:(tcn + 1) * 128, :],
                                  in_=yv[:])

            if "2" in _PHASES:
                LOOK = 2   # emit score pairs ahead so PE's in-order queue
                done_tc = 0  # never stalls on the exp/mask chain
                for gi, g in enumerate(GROUPS):
                    for gg in GROUPS[gi:gi + 1 + LOOK]:
                        for _c, ti, _to, _oo, _ln in gg["segs"]:
                            if ti not in emitted:
                                emit_pair(ti)
                    do_group(g)
                    # normalize runs one group behind so the bcast/mul never
                    # wait mid-queue on this group's recip
                    if gi > 0:
                        gp = GROUPS[gi - 1]
                        normalize(gp)
                        rows_done = gp["rows"][-1] + 1
                        while ("4" in _PHASES and done_tc < 32
                               and rows_done >= (done_tc + 1) * 2):
                            project(done_tc)
                            done_tc += 1
                normalize(GROUPS[-1])
                while "4" in _PHASES and done_tc < 32:
                    project(done_tc)
                    done_tc += 1
    nc.compile()
    return nc


def _get_module():
    if "nc" not in _NC_CACHE:
        _NC_CACHE["nc"] = _build_module()
    return _NC_CACHE["nc"]


# ---------------------------------------------------------------- host

def kernel(x, w_qkv, w_out):
    x = np.asarray(x, np.float32)
    w_qkv = np.asarray(w_qkv, np.float32)
    w_out = np.asarray(w_out, np.float32)
    nc = _get_module()

    bf = ml_dtypes.bfloat16
    masks_pair = np.ascontiguousarray(
        np.broadcast_to(MASKS[:, :, None, :], (NMASK, 128, 2, 512))).astype(bf)
    xT = [np.ascontiguousarray(x[b].reshape(S, D).T).astype(bf) for b in range(B)]
    w_outT = np.ascontiguousarray(w_out.T)

    in_maps = []
    for c in range(NCORES):
        b, h0 = c // 4, 2 * (c % 4)
        f = h0 * 64
        in_maps.append({
            "xT": xT[b],
            "wq": np.ascontiguousarray(w_qkv[f:f + 128].T).astype(bf),
            "wk": np.ascontiguousarray(w_qkv[512 + f:512 + f + 128].T).astype(bf),
            "wv": np.ascontiguousarray(w_qkv[1024 + f:1024 + f + 128].T).astype(bf),
            "wo": np.ascontiguousarray(w_outT[f:f + 128]).astype(bf),
            "masks": masks_pair,
        })
    res = run_bass_kernel_spmd(nc, in_maps, list(range(NCORES)), trace=TRACE)
    global LAST_RESULTS
    LAST_RESULTS = res
    y = np.zeros((B, S, D), np.float32)
    for c in range(NCORES):
        y[c // 4] += res.results[c]["y"].astype(np.float32)
    return y.reshape(B, H, W, D)


# revision 46
# speedup vs baseline: 1.0200x; 1.0200x over previous
"""Neighbourhood attention block (7x7 clamped window) on 8 Trainium2 cores.

Sharding: (batch, head-pair) tensor parallel. Core c handles batch b = c//4
and heads (2*(c%4), 2*(c%4)+1). Each core computes q/k/v projections for its
two heads, neighbourhood attention, and a partial output projection
y_partial = ao @ w_out_slice^T (bf16). Host sums the 4 partials per batch.

v2 layout (vs baseline): everything bf16 off-host (halves DMA + enables DVE
fast modes), input DMA chunked so projections start early, scores for the two
heads paired into one PSUM tile (one exp + one mask-mul per chunk-tile pair),
denominator rides the PV staging copy, normalize done per 512-token chunk
(reciprocal + partition_broadcast + bf16 mul) feeding the output projection
pipeline.

Attention layout: scoresT [key, query] tiles so PV needs no transposes.
Keys are chunked 2 image rows (128 tokens) per chunk; each chunk is matched
against the 8 query rows that can see it (512 queries). Masking is
multiplicative 0/1 after exp (bf16), so invalid keys contribute 0 to both the
PV numerator and the denominator (a ones-column appended to V).
"""
import os
import numpy as np
import ml_dtypes
from contextlib import ExitStack

_PHASES = os.environ.get("KERNEL_PHASES", "1234")  # debug bisect knob
_ST_ROT = int(os.environ.get("KOPT_ST", "0"))      # st copies: 0 fixed, 1 rotate
_PVP = int(os.environ.get("KOPT_PVP", "2"))        # pv pool bufs
_OPP = int(os.environ.get("KOPT_OPP", "2"))        # out-proj pool bufs
_LOOK = int(os.environ.get("KOPT_LOOK", "1"))      # score-pair lookahead (groups)

import concourse.bass as bass
import concourse.bacc as bacc
import concourse.tile as tile
import concourse.mybir as mybir
from concourse.bass_utils import run_bass_kernel_spmd
from concourse.masks import make_identity

F32 = mybir.dt.float32
BF16 = mybir.dt.bfloat16

B, H, W, D = 2, 64, 64, 512
DH, NH = 64, 8
S = H * W              # 4096 tokens per batch
KER = 7
SCALE = DH ** -0.5     # 0.125
NCORES = 8

# ---------------------------------------------------------------- geometry

def _sh(r):            # clamped window start (rows); same formula for cols
    return min(max(r - KER // 2, 0), H - KER)


def _chunks_of_row(r):  # key chunks (2 rows each) seen by query row r
    s = _sh(r)
    return list(range(s // 2, (s + KER + 1) // 2))


def _build_plan():
    """Tiles: scoresT [128 keys of chunk c, qw queries at q0]. Groups: PV
    accumulations [65, qw] covering disjoint query ranges."""
    tiles = []          # dict(c, q0, qw)
    for c in range(32):
        q0r = min(max(2 * c - 3, 0), 56)
        tiles.append(dict(c=c, q0=q0r * 64, qw=512))
    for c in (2, 3):        # query rows 0..2 miss these chunks' main windows
        tiles.append(dict(c=c, q0=0, qw=192))
    for c in (28, 29):      # query rows 61..63
        tiles.append(dict(c=c, q0=61 * 64, qw=192))

    # rows covered by each tile, for resolution
    def covers(t, r):
        return t["q0"] <= r * 64 and (r + 1) * 64 <= t["q0"] + t["qw"]

    groups = []         # dict(q0, qw, rows)
    groups.append(dict(rows=[0]))
    groups.append(dict(rows=[1, 2]))
    for k in range(14):
        r0 = 4 * k + 3
        groups.append(dict(rows=[r0, r0 + 1, r0 + 2, r0 + 3]))
    groups.append(dict(rows=[59, 60, 61, 62]))
    groups.append(dict(rows=[63]))

    for g in groups:
        rows = g["rows"]
        g["q0"] = rows[0] * 64
        g["qw"] = len(rows) * 64
        # chunk -> contiguous row subrange of this group needing it
        chunk_rows = {}
        for r in rows:
            for c in _chunks_of_row(r):
                a, b = chunk_rows.get(c, (r, r))
                chunk_rows[c] = (min(a, r), max(b, r))
        mms = []        # (c, row_a, row_b_inclusive, width)
        for c, (ra, rb) in sorted(chunk_rows.items()):
            mms.append((c, ra, rb, (rb - ra + 1) * 64))
        mms.sort(key=lambda m: -m[3])   # widest (full-group) first for start=True
        assert mms[0][3] == g["qw"], (g, mms)
        # resolve each (c, row range) to probs-tile segments
        segs = []       # (c, tile_i, tile_off, out_off, length)
        for c, ra, rb, _w in mms:
            r = ra
            while r <= rb:
                cand = [i for i, t in enumerate(tiles) if t["c"] == c and covers(t, r)]
                assert cand, (g, c, r)
                ti = cand[0]
                t = tiles[ti]
                # extend run while same tile covers
                r2 = r
                while r2 + 1 <= rb and covers(t, r2 + 1):
                    r2 += 1
                segs.append((c, ti, r * 64 - t["q0"], r * 64 - g["q0"],
                             (r2 - r + 1) * 64))
                r = r2 + 1
        g["segs"] = segs

    # sanity: every (query row, chunk) incidence consumed exactly once
    seen = set()
    for g in groups:
        for c, ti, toff, ooff, ln in g["segs"]:
            for r in range((g["q0"] + ooff) // 64, (g["q0"] + ooff + ln) // 64):
                key = (r, c)
                assert key not in seen, key
                seen.add(key)
    for r in range(H):
        for c in _chunks_of_row(r):
            assert (r, c) in seen, (r, c)

    # masks per tile (0/1), deduped
    starts = np.minimum(np.maximum(np.arange(H) - KER // 2, 0), H - KER)
    valid = (np.arange(H)[None, :] >= starts[:, None]) & \
            (np.arange(H)[None, :] < starts[:, None] + KER)   # [q pos, k pos]

    def tile_mask(t):
        ktok = t["c"] * 128 + np.arange(128)
        qtok = t["q0"] + np.arange(t["qw"])
        m = np.zeros((128, 512), np.float32)
        m[:, :t["qw"]] = (valid[qtok[None, :] // 64, ktok[:, None] // 64]
                          & valid[qtok[None, :] % 64, ktok[:, None] % 64])
        return m

    mask_list, mask_ids = [], {}
    for t in tiles:
        m = tile_mask(t)
        key = m.tobytes()
        if key not in mask_ids:
            mask_ids[key] = len(mask_list)
            mask_list.append(m)
        t["mask_id"] = mask_ids[key]
    return tiles, groups, np.stack(mask_list)


TILES, GROUPS, MASKS = _build_plan()
NMASK = len(MASKS)

# ---------------------------------------------------------------- device

_NC_CACHE = {}
TRACE = False          # set True (e.g. from test.py) to capture an NTFF profile
LAST_RESULTS = None    # BassKernelResults of the most recent kernel() call


def _build_module():
    nc = bacc.Bacc("TRN2", target_bir_lowering=False, debug=False,
                   num_devices=NCORES)
    xT_d = nc.dram_tensor("xT", [D, S], BF16, kind="ExternalInput")
    wq_d = nc.dram_tensor("wq", [D, 128], BF16, kind="ExternalInput")
    wk_d = nc.dram_tensor("wk", [D, 128], BF16, kind="ExternalInput")
    wv_d = nc.dram_tensor("wv", [D, 128], BF16, kind="ExternalInput")
    wo_d = nc.dram_tensor("wo", [128, 512], BF16, kind="ExternalInput")
    mk_d = nc.dram_tensor("masks", [NMASK, 128, 2, 512], BF16,
                          kind="ExternalInput")
    y_d = nc.dram_tensor("y", [S, D], BF16, kind="ExternalOutput")

    NB = 8              # 512-token projection chunks

    with tile.TileContext(nc) as tc, ExitStack() as ctx:
        const = ctx.enter_context(tc.tile_pool(name="const", bufs=1))
        # DMA order tracks first-use: wq + first x chunk gate the first
        # projection; wo + masks aren't needed until attention starts.
        wq_t = const.tile([128, 4, 128], BF16, tag="wq")
        nc.sync.dma_start(out=wq_t[:], in_=wq_d.ap().rearrange("(c p) m -> p c m", p=128))
        xT_t = const.tile([128, 4, S], BF16, tag="xT")
        CB0 = S // NB
        nc.sync.dma_start(out=xT_t[:, :, 0:CB0],
                          in_=xT_d.ap()[:, 0:CB0].rearrange("(c p) t -> p c t", p=128))
        wk_t = const.tile([128, 4, 128], BF16, tag="wk")
        nc.sync.dma_start(out=wk_t[:], in_=wk_d.ap().rearrange("(c p) m -> p c m", p=128))
        wv_t = const.tile([128, 4, 128], BF16, tag="wv")
        nc.sync.dma_start(out=wv_t[:], in_=wv_d.ap().rearrange("(c p) m -> p c m", p=128))
        for nb in range(1, NB):
            sl = slice(nb * CB0, (nb + 1) * CB0)
            nc.sync.dma_start(
                out=xT_t[:, :, sl],
                in_=xT_d.ap()[:, sl].rearrange("(c p) t -> p c t", p=128))
        wo_t = const.tile([128, 512], BF16, tag="wo")
        nc.sync.dma_start(out=wo_t[:], in_=wo_d[:, :])
        mk_t = const.tile([128, NMASK, 2, 512], BF16, tag="mk")
        nc.sync.dma_start(out=mk_t[:], in_=mk_d.ap().rearrange("m p c w -> p m c w"))

        qT = const.tile([128, S], BF16, tag="qT")      # [2 heads x 64e, tok]
        kT = const.tile([128, S], BF16, tag="kT")
        vT = const.tile([128, S], BF16, tag="vT")
        # V: [tok_in_chunk, chunk, 130]: cols 0:64 u0-e, 64 ones, 65:129 u1-e, 129 ones
        V = const.tile([128, 32, 130], BF16, tag="V")
        nc.gpsimd.memset(V[:, :, 64:65], 1.0)   # only the ones-columns; the
        nc.gpsimd.memset(V[:, :, 129:130], 1.0)  # rest is overwritten below
        st = const.tile([65, 2, S], BF16, tag="st")    # unnormalized pv + denom row
        rec = const.tile([1, 2, S], BF16, tag="rec")   # 1/denom per unit
        rb0 = const.tile([64, S], BF16, tag="rb0")     # recip broadcast per unit
        rb1 = const.tile([64, S], BF16, tag="rb1")
        aoT = const.tile([128, S], BF16, tag="aoT")
        ident = const.tile([128, 128], BF16, tag="ident")
        make_identity(nc, ident[:])

        def copy_on(eng, dst, src):
            if eng is nc.scalar:
                eng.activation(dst, src, mybir.ActivationFunctionType.Copy)
            else:
                eng.tensor_copy(dst, src)

        cp_i = 0

        def copy(dst, src, engs=(nc.vector, nc.scalar, nc.gpsimd)):
            nonlocal cp_i
            eng = engs[cp_i % len(engs)]
            cp_i += 1
            copy_on(eng, dst, src)

        # ---- phase 1: projections
        CB = S // NB
        with tc.tile_pool(name="pps", bufs=3, space="PSUM") as pps:
            for nb in range(NB) if "1" in _PHASES else ():
                sl = slice(nb * CB, (nb + 1) * CB)
                for w_t, dst in ((wq_t, qT), (wk_t, kT), (wv_t, vT)):
                    acc = pps.tile([128, 512], F32, tag="acc")
                    for dc in range(4):
                        nc.tensor.matmul(acc[:], w_t[:, dc, :],
                                         xT_t[:, dc, sl],
                                         start=(dc == 0), stop=(dc == 3))
                    copy(dst[:, sl], acc[:],
                         engs=(nc.vector, nc.vector, nc.scalar))
                # V transpose for the 4 chunks of this nb (vT cols now ready)
                for pci in range(nb * 2, nb * 2 + 2):
                    tp = pps.tile([128, 2, 128], BF16, tag="tp")
                    for s in range(2):
                        ci = pci * 2 + s
                        nc.tensor.transpose(tp[:, s, :],
                                            vT[:, ci * 128:(ci + 1) * 128],
                                            ident[:])
                    # gpsimd cannot read PSUM: these stay on DVE/Act
                    copy(V[:, pci * 2:pci * 2 + 2, 0:64], tp[:, :, 0:64],
                         engs=(nc.vector, nc.scalar))
                    copy(V[:, pci * 2:pci * 2 + 2, 65:129], tp[:, :, 64:128],
                         engs=(nc.vector, nc.scalar))

        # ---- phase 2+3+4: attention, normalize, output projection
        # normalize chunk cn (512 tokens = 8 query rows) is emitted as soon as
        # the groups covering those rows are done; output projection follows.
        with tc.tile_pool(name="scp", bufs=2, space="PSUM") as scp, \
             tc.tile_pool(name="pvp", bufs=_PVP, space="PSUM") as pvp, \
             tc.tile_pool(name="opp", bufs=_OPP, space="PSUM") as opp, \
             tc.tile_pool(name="prp", bufs=10) as prp, \
             tc.tile_pool(name="yvp", bufs=4) as yvp:
            emitted = {}

            def emit_pair(ti):
                t = TILES[ti]
                qw, c = t["qw"], t["c"]
                sc = scp.tile([128, 2, 512], F32, tag="sc")
                for u in (0, 1):
                    ue = slice(u * 64, u * 64 + 64)
                    nc.tensor.matmul(sc[:, u, :qw],
                                     kT[ue, c * 128:(c + 1) * 128],
                                     qT[ue, t["q0"]:t["q0"] + qw],
                                     start=True, stop=True)
                pr = prp.tile([128, 2, 512], BF16, tag="pr")
                nc.scalar.activation(pr[:, :, :qw], sc[:, :, :qw],
                                     mybir.ActivationFunctionType.Exp,
                                     scale=SCALE)
                if qw == 512:   # flat 2D APs qualify for DVE fast modes
                    nc.vector.tensor_mul(
                        pr[:].rearrange("p c w -> p (c w)"),
                        pr[:].rearrange("p c w -> p (c w)"),
                        mk_t[:, t["mask_id"]].rearrange("p c w -> p (c w)"))
                else:
                    nc.vector.tensor_mul(pr[:, :, :qw], pr[:, :, :qw],
                                         mk_t[:, t["mask_id"], :, :qw])
                emitted[ti] = pr

            def do_group(g):
                qw = g["qw"]
                sl = slice(g["q0"], g["q0"] + qw)
                nseg = len(g["segs"])
                pv = pvp.tile([65, 2, 256], F32, tag="pv")   # both units, 1 bank
                for u in (0, 1):
                    uv = slice(u * 65, u * 65 + 65)
                    for si, (c, ti, toff, ooff, ln) in enumerate(g["segs"]):
                        nc.tensor.matmul(pv[:, u, ooff:ooff + ln],
                                         V[:, c, uv],
                                         emitted[ti][:, u, toff:toff + ln],
                                         start=(si == 0), stop=(si == nseg - 1))
                if _ST_ROT:
                    copy(st[:, 0, sl], pv[:, 0, :qw], engs=(nc.vector, nc.scalar))
                    copy(st[:, 1, sl], pv[:, 1, :qw], engs=(nc.vector, nc.scalar))
                else:
                    copy_on(nc.vector, st[:, 0, sl], pv[:, 0, :qw])
                    copy_on(nc.scalar, st[:, 1, sl], pv[:, 1, :qw])
                with nc.allow_low_precision(reason="denominators are O(1)-O(1e3); bf16 recip ~0.4% rel err, gate is 2e-2"):
                    nc.vector.reciprocal(rec[:, :, sl], st[64:65, :, sl])

            def normalize(g):
                sl = slice(g["q0"], g["q0"] + g["qw"])
                _mul_engs = ((nc.vector, nc.gpsimd), (nc.gpsimd, nc.gpsimd),
                             (nc.vector, nc.vector))[int(os.environ.get("KOPT_MUL", "0"))]
                for u, rb in ((0, rb0), (1, rb1)):
                    ud = slice(u * 64, u * 64 + 64)
                    nc.gpsimd.partition_broadcast(rb[:, sl], rec[0:1, u, sl])
                    # all-SBUF bf16 mul: legal on gpsimd; spread across engines
                    _mul_engs[u].tensor_mul(aoT[ud, sl], st[0:64, u, sl],
                                            rb[:, sl])

            def project(tcn):     # one 128-token chunk: matmul, stage, DMA out
                acc = opp.tile([128, 512], F32, tag="oacc")
                nc.tensor.matmul(acc[:], aoT[:, tcn * 128:(tcn + 1) * 128],
                                 wo_t[:], start=True, stop=True)
                yv = yvp.tile([128, 512], BF16, tag="yv")
                copy(yv[:], acc[:], engs=(nc.scalar, nc.vector))
                nc.sync.dma_start(out=y_d[tcn * 128:(tcn + 1) * 128, :],
                                  in_=yv[:])

            if "2" in _PHASES:
                LOOK = _LOOK  # emit score pairs ahead so PE's in-order queue
                done_tc = 0   # never stalls on the exp/mask chain
                for gi, g in enumerate(GROUPS):
                    for gg in GROUPS[gi:gi + 1 + LOOK]:
                        for _c, ti, _to, _oo, _ln in gg["segs"]:
                            if ti not in emitted:
                                emit_pair(ti)
                    do_group(g)
                    # normalize runs one group behind so the bcast/mul never
                    # wait mid-queue on this group's recip
                    if gi > 0:
                        gp = GROUPS[gi - 1]
                        normalize(gp)
                        rows_done = gp["rows"][-1] + 1
                        while ("4" in _PHASES and done_tc < 32
                               and rows_done >= (done_tc + 1) * 2):
                            project(done_tc)
                            done_tc += 1
                normalize(GROUPS[-1])
                while "4" in _PHASES and done_tc < 32:
                    project(done_tc)
                    done_tc += 1
    nc.compile()
    return nc


def _get_module():
    if "nc" not in _NC_CACHE:
        _NC_CACHE["nc"] = _build_module()
    return _NC_CACHE["nc"]


# ---------------------------------------------------------------- host

def kernel(x, w_qkv, w_out):
    x = np.asarray(x, np.float32)
    w_qkv = np.asarray(w_qkv, np.float32)
    w_out = np.asarray(w_out, np.float32)
    nc = _get_module()

    bf = ml_dtypes.bfloat16
    masks_pair = np.ascontiguousarray(
        np.broadcast_to(MASKS[:, :, None, :], (NMASK, 128, 2, 512))).astype(bf)
    xT = [np.ascontiguousarray(x[b].reshape(S, D).T).astype(bf) for b in range(B)]
    w_outT = np.ascontiguousarray(w_out.T)

    in_maps = []
    for c in range(NCORES):
        b, h0 = c // 4, 2 * (c % 4)
        f = h0 * 64
        in_maps.append({
            "xT": xT[b],
            "wq": np.ascontiguousarray(w_qkv[f:f + 128].T).astype(bf),
            "wk": np.ascontiguousarray(w_qkv[512 + f:512 + f + 128].T).astype(bf),
            "wv": np.ascontiguousarray(w_qkv[1024 + f:1024 + f + 128].T).astype(bf),
            "wo": np.ascontiguousarray(w_outT[f:f + 128]).astype(bf),
            "masks": masks_pair,
        })
    res = run_bass_kernel_spmd(nc, in_maps, list(range(NCORES)), trace=TRACE)
    global LAST_RESULTS
    LAST_RESULTS = res
    y = np.zeros((B, S, D), np.float32)
    for c in range(NCORES):
        y[c // 4] += res.results[c]["y"].astype(np.float32)
    return y.reshape(B, H, W, D)


# revision 53
# speedup vs baseline: 1.0796x; 1.0584x over previous
"""Neighbourhood attention block (7x7 clamped window) on 8 Trainium2 cores.

Sharding: (batch, head-pair) tensor parallel. Core c handles batch b = c//4
and heads (2*(c%4), 2*(c%4)+1). Each core computes q/k/v projections for its
two heads, neighbourhood attention, and a partial output projection
y_partial = ao @ w_out_slice^T (bf16). Host sums the 4 partials per batch.

v2 layout (vs baseline): everything bf16 off-host (halves DMA + enables DVE
fast modes), input DMA chunked so projections start early, scores for the two
heads paired into one PSUM tile (one exp + one mask-mul per chunk-tile pair),
denominator rides the PV staging copy, normalize done per 512-token chunk
(reciprocal + partition_broadcast + bf16 mul) feeding the output projection
pipeline.

Attention layout: scoresT [key, query] tiles so PV needs no transposes.
Keys are chunked 2 image rows (128 tokens) per chunk; each chunk is matched
against the 8 query rows that can see it (512 queries). Masking is
multiplicative 0/1 after exp (bf16), so invalid keys contribute 0 to both the
PV numerator and the denominator (a ones-column appended to V).
"""
import os
import numpy as np
import ml_dtypes
from contextlib import ExitStack

_PHASES = os.environ.get("KERNEL_PHASES", "1234")  # debug bisect knob
_ST_ROT = int(os.environ.get("KOPT_ST", "0"))      # st copies: 0 fixed, 1 rotate
_PVP = int(os.environ.get("KOPT_PVP", "1"))        # pv pool bufs
_OPP = int(os.environ.get("KOPT_OPP", "2"))        # out-proj pool bufs
_LOOK = int(os.environ.get("KOPT_LOOK", "2"))      # score-pair lookahead (groups)

import concourse.bass as bass
import concourse.bacc as bacc
import concourse.tile as tile
import concourse.mybir as mybir
from concourse.bass_utils import run_bass_kernel_spmd
from concourse.masks import make_identity

F32 = mybir.dt.float32
BF16 = mybir.dt.bfloat16

B, H, W, D = 2, 64, 64, 512
DH, NH = 64, 8
S = H * W              # 4096 tokens per batch
KER = 7
SCALE = DH ** -0.5     # 0.125
NCORES = 8

# ---------------------------------------------------------------- geometry

def _sh(r):            # clamped window start (rows); same formula for cols
    return min(max(r - KER // 2, 0), H - KER)


def _chunks_of_row(r):  # key chunks (2 rows each) seen by query row r
    s = _sh(r)
    return list(range(s // 2, (s + KER + 1) // 2))


def _build_plan():
    """Tiles: scoresT [128 keys of chunk c, qw queries at q0]. Groups: PV
    accumulations [65, qw] covering disjoint query ranges."""
    tiles = []          # dict(c, q0, qw)
    for c in range(32):
        q0r = min(max(2 * c - 3, 0), 56)
        tiles.append(dict(c=c, q0=q0r * 64, qw=512))
    for c in (2, 3):        # query rows 0..2 miss these chunks' main windows
        tiles.append(dict(c=c, q0=0, qw=192))
    for c in (28, 29):      # query rows 61..63
        tiles.append(dict(c=c, q0=61 * 64, qw=192))

    # rows covered by each tile, for resolution
    def covers(t, r):
        return t["q0"] <= r * 64 and (r + 1) * 64 <= t["q0"] + t["qw"]

    groups = []         # dict(q0, qw, rows)
    groups.append(dict(rows=[0]))
    groups.append(dict(rows=[1, 2]))
    for k in range(14):
        r0 = 4 * k + 3
        groups.append(dict(rows=[r0, r0 + 1, r0 + 2, r0 + 3]))
    groups.append(dict(rows=[59, 60, 61, 62]))
    groups.append(dict(rows=[63]))

    for g in groups:
        rows = g["rows"]
        g["q0"] = rows[0] * 64
        g["qw"] = len(rows) * 64
        # chunk -> contiguous row subrange of this group needing it
        chunk_rows = {}
        for r in rows:
            for c in _chunks_of_row(r):
                a, b = chunk_rows.get(c, (r, r))
                chunk_rows[c] = (min(a, r), max(b, r))
        mms = []        # (c, row_a, row_b_inclusive, width)
        for c, (ra, rb) in sorted(chunk_rows.items()):
            mms.append((c, ra, rb, (rb - ra + 1) * 64))
        mms.sort(key=lambda m: -m[3])   # widest (full-group) first for start=True
        assert mms[0][3] == g["qw"], (g, mms)
        # resolve each (c, row range) to probs-tile segments
        segs = []       # (c, tile_i, tile_off, out_off, length)
        for c, ra, rb, _w in mms:
            r = ra
            while r <= rb:
                cand = [i for i, t in enumerate(tiles) if t["c"] == c and covers(t, r)]
                assert cand, (g, c, r)
                ti = cand[0]
                t = tiles[ti]
                # extend run while same tile covers
                r2 = r
                while r2 + 1 <= rb and covers(t, r2 + 1):
                    r2 += 1
                segs.append((c, ti, r * 64 - t["q0"], r * 64 - g["q0"],
                             (r2 - r + 1) * 64))
                r = r2 + 1
        g["segs"] = segs

    # sanity: every (query row, chunk) incidence consumed exactly once
    seen = set()
    for g in groups:
        for c, ti, toff, ooff, ln in g["segs"]:
            for r in range((g["q0"] + ooff) // 64, (g["q0"] + ooff + ln) // 64):
                key = (r, c)
                assert key not in seen, key
                seen.add(key)
    for r in range(H):
        for c in _chunks_of_row(r):
            assert (r, c) in seen, (r, c)

    # masks per tile (0/1), deduped
    starts = np.minimum(np.maximum(np.arange(H) - KER // 2, 0), H - KER)
    valid = (np.arange(H)[None, :] >= starts[:, None]) & \
            (np.arange(H)[None, :] < starts[:, None] + KER)   # [q pos, k pos]

    def tile_mask(t):
        ktok = t["c"] * 128 + np.arange(128)
        qtok = t["q0"] + np.arange(t["qw"])
        m = np.zeros((128, 512), np.float32)
        m[:, :t["qw"]] = (valid[qtok[None, :] // 64, ktok[:, None] // 64]
                          & valid[qtok[None, :] % 64, ktok[:, None] % 64])
        return m

    mask_list, mask_ids = [], {}
    for t in tiles:
        m = tile_mask(t)
        key = m.tobytes()
        if key not in mask_ids:
            mask_ids[key] = len(mask_list)
            mask_list.append(m)
        t["mask_id"] = mask_ids[key]
    return tiles, groups, np.stack(mask_list)


TILES, GROUPS, MASKS = _build_plan()
NMASK = len(MASKS)

# ---------------------------------------------------------------- device

_NC_CACHE = {}
TRACE = False          # set True (e.g. from test.py) to capture an NTFF profile
LAST_RESULTS = None    # BassKernelResults of the most recent kernel() call


def _build_module():
    nc = bacc.Bacc("TRN2", target_bir_lowering=False, debug=False,
                   num_devices=NCORES)
    xT_d = nc.dram_tensor("xT", [D, S], BF16, kind="ExternalInput")
    wq_d = nc.dram_tensor("wq", [D, 128], BF16, kind="ExternalInput")
    wk_d = nc.dram_tensor("wk", [D, 128], BF16, kind="ExternalInput")
    wv_d = nc.dram_tensor("wv", [D, 128], BF16, kind="ExternalInput")
    wo_d = nc.dram_tensor("wo", [128, 512], BF16, kind="ExternalInput")
    mk_d = nc.dram_tensor("masks", [NMASK, 128, 2, 512], BF16,
                          kind="ExternalInput")
    y_d = nc.dram_tensor("y", [S, D], BF16, kind="ExternalOutput")

    NB = 8              # 512-token projection chunks

    with tile.TileContext(nc) as tc, ExitStack() as ctx:
        const = ctx.enter_context(tc.tile_pool(name="const", bufs=1))
        # DMA order tracks first-use: wq + first x chunk gate the first
        # projection; wo + masks aren't needed until attention starts.
        wq_t = const.tile([128, 4, 128], BF16, tag="wq")
        nc.sync.dma_start(out=wq_t[:], in_=wq_d.ap().rearrange("(c p) m -> p c m", p=128))
        xT_t = const.tile([128, 4, S], BF16, tag="xT")
        CB0 = S // NB
        nc.sync.dma_start(out=xT_t[:, :, 0:CB0],
                          in_=xT_d.ap()[:, 0:CB0].rearrange("(c p) t -> p c t", p=128))
        wk_t = const.tile([128, 4, 128], BF16, tag="wk")
        nc.sync.dma_start(out=wk_t[:], in_=wk_d.ap().rearrange("(c p) m -> p c m", p=128))
        wv_t = const.tile([128, 4, 128], BF16, tag="wv")
        nc.sync.dma_start(out=wv_t[:], in_=wv_d.ap().rearrange("(c p) m -> p c m", p=128))
        for nb in range(1, NB):
            sl = slice(nb * CB0, (nb + 1) * CB0)
            nc.sync.dma_start(
                out=xT_t[:, :, sl],
                in_=xT_d.ap()[:, sl].rearrange("(c p) t -> p c t", p=128))
        wo_t = const.tile([128, 512], BF16, tag="wo")
        nc.sync.dma_start(out=wo_t[:], in_=wo_d[:, :])
        mk_t = const.tile([128, NMASK, 2, 512], BF16, tag="mk")
        nc.sync.dma_start(out=mk_t[:], in_=mk_d.ap().rearrange("m p c w -> p m c w"))

        qT = const.tile([128, S], BF16, tag="qT")      # [2 heads x 64e, tok]
        kT = const.tile([128, S], BF16, tag="kT")
        vT = const.tile([128, S], BF16, tag="vT")
        # V: [tok_in_chunk, chunk, 130]: cols 0:64 u0-e, 64 ones, 65:129 u1-e, 129 ones
        V = const.tile([128, 32, 130], BF16, tag="V")
        nc.gpsimd.memset(V[:, :, 64:65], 1.0)   # only the ones-columns; the
        nc.gpsimd.memset(V[:, :, 129:130], 1.0)  # rest is overwritten below
        st = const.tile([65, 2, S], BF16, tag="st")    # unnormalized pv + denom row
        rec = const.tile([1, 2, S], BF16, tag="rec")   # 1/denom per unit
        rb0 = const.tile([64, S], BF16, tag="rb0")     # recip broadcast per unit
        rb1 = const.tile([64, S], BF16, tag="rb1")
        aoT = const.tile([128, S], BF16, tag="aoT")
        ident = const.tile([128, 128], BF16, tag="ident")
        make_identity(nc, ident[:])

        def copy_on(eng, dst, src):
            if eng is nc.scalar:
                eng.activation(dst, src, mybir.ActivationFunctionType.Copy)
            else:
                eng.tensor_copy(dst, src)

        cp_i = 0

        def copy(dst, src, engs=(nc.vector, nc.scalar, nc.gpsimd)):
            nonlocal cp_i
            eng = engs[cp_i % len(engs)]
            cp_i += 1
            copy_on(eng, dst, src)

        # PSUM pool layout: the attention pools (scp/pvp) open FIRST so their
        # banks carry no dependency on late phase-1 tiles; pps (phase 1, one
        # 2KB tag — transposes write into bitcast views of acc slots) takes
        # the rest and hands its banks to opp when it closes.
        CB = S // NB
        with tc.tile_pool(name="scp", bufs=2, space="PSUM") as scp, \
             tc.tile_pool(name="pvp", bufs=_PVP, space="PSUM") as pvp, \
             tc.tile_pool(name="prp", bufs=10) as prp, \
             tc.tile_pool(name="yvp", bufs=4) as yvp:
            with tc.tile_pool(name="pps", bufs=3, space="PSUM") as pps:
                for nb in range(NB) if "1" in _PHASES else ():
                    sl = slice(nb * CB, (nb + 1) * CB)
                    for w_t, dst in ((wq_t, qT), (wk_t, kT), (wv_t, vT)):
                        acc = pps.tile([128, 512], F32, tag="acc")
                        for dc in range(4):
                            nc.tensor.matmul(acc[:], w_t[:, dc, :],
                                             xT_t[:, dc, sl],
                                             start=(dc == 0), stop=(dc == 3))
                        _p = int(os.environ.get("KOPT_PH1", "0"))
                        copy(dst[:, sl], acc[:],
                             engs=((nc.vector, nc.vector, nc.scalar),
                                   (nc.vector,),
                                   (nc.vector, nc.scalar))[_p])
                    # V transpose for the 4 chunks of this nb (vT ready)
                    for pci in range(nb * 2, nb * 2 + 2):
                        tpa = pps.tile([128, 512], F32, tag="acc")
                        tp = tpa[:].bitcast(BF16).rearrange(
                            "p (c e) -> p c e", c=2)[:, :, 0:128]
                        for s in range(2):
                            ci = pci * 2 + s
                            nc.tensor.transpose(tp[:, s, :],
                                                vT[:, ci * 128:(ci + 1) * 128],
                                                ident[:])
                        # gpsimd cannot read PSUM: these stay on DVE/Act
                        copy(V[:, pci * 2:pci * 2 + 2, 0:64], tp[:, :, 0:64],
                             engs=(nc.vector, nc.scalar))
                        copy(V[:, pci * 2:pci * 2 + 2, 65:129],
                             tp[:, :, 64:128], engs=(nc.vector, nc.scalar))
            opp_cm = tc.tile_pool(name="opp", bufs=_OPP, space="PSUM")
            opp = opp_cm.__enter__()
            emitted = {}

            def emit_pair(ti):
                t = TILES[ti]
                qw, c = t["qw"], t["c"]
                sc = scp.tile([128, 2, 512], F32, tag="sc")
                for u in (0, 1):
                    ue = slice(u * 64, u * 64 + 64)
                    nc.tensor.matmul(sc[:, u, :qw],
                                     kT[ue, c * 128:(c + 1) * 128],
                                     qT[ue, t["q0"]:t["q0"] + qw],
                                     start=True, stop=True)
                pr = prp.tile([128, 2, 512], BF16, tag="pr")
                nc.scalar.activation(pr[:, :, :qw], sc[:, :, :qw],
                                     mybir.ActivationFunctionType.Exp,
                                     scale=SCALE)
                if qw == 512:   # flat 2D APs qualify for DVE fast modes
                    nc.vector.tensor_mul(
                        pr[:].rearrange("p c w -> p (c w)"),
                        pr[:].rearrange("p c w -> p (c w)"),
                        mk_t[:, t["mask_id"]].rearrange("p c w -> p (c w)"))
                else:
                    nc.vector.tensor_mul(pr[:, :, :qw], pr[:, :, :qw],
                                         mk_t[:, t["mask_id"], :, :qw])
                emitted[ti] = pr

            def do_group(g):
                qw = g["qw"]
                sl = slice(g["q0"], g["q0"] + qw)
                nseg = len(g["segs"])
                pv = pvp.tile([65, 2, 256], F32, tag="pv")   # both units, 1 bank
                for u in (0, 1):
                    uv = slice(u * 65, u * 65 + 65)
                    for si, (c, ti, toff, ooff, ln) in enumerate(g["segs"]):
                        nc.tensor.matmul(pv[:, u, ooff:ooff + ln],
                                         V[:, c, uv],
                                         emitted[ti][:, u, toff:toff + ln],
                                         start=(si == 0), stop=(si == nseg - 1))
                if _ST_ROT:
                    copy(st[:, 0, sl], pv[:, 0, :qw], engs=(nc.vector, nc.scalar))
                    copy(st[:, 1, sl], pv[:, 1, :qw], engs=(nc.vector, nc.scalar))
                else:
                    copy_on(nc.vector, st[:, 0, sl], pv[:, 0, :qw])
                    copy_on(nc.scalar, st[:, 1, sl], pv[:, 1, :qw])
                with nc.allow_low_precision(reason="denominators are O(1)-O(1e3); bf16 recip ~0.4% rel err, gate is 2e-2"):
                    nc.vector.reciprocal(rec[:, :, sl], st[64:65, :, sl])

            def normalize(g):
                sl = slice(g["q0"], g["q0"] + g["qw"])
                _mul_engs = ((nc.vector, nc.gpsimd), (nc.gpsimd, nc.gpsimd),
                             (nc.vector, nc.vector))[int(os.environ.get("KOPT_MUL", "0"))]
                for u, rb in ((0, rb0), (1, rb1)):
                    ud = slice(u * 64, u * 64 + 64)
                    nc.gpsimd.partition_broadcast(rb[:, sl], rec[0:1, u, sl])
                    # all-SBUF bf16 mul: legal on gpsimd; spread across engines
                    _mul_engs[u].tensor_mul(aoT[ud, sl], st[0:64, u, sl],
                                            rb[:, sl])

            def project(tcn):     # one 128-token chunk: matmul, stage, DMA out
                acc = opp.tile([128, 512], F32, tag="oacc")
                nc.tensor.matmul(acc[:], aoT[:, tcn * 128:(tcn + 1) * 128],
                                 wo_t[:], start=True, stop=True)
                yv = yvp.tile([128, 512], BF16, tag="yv")
                copy(yv[:], acc[:], engs=(nc.scalar, nc.vector))
                nc.sync.dma_start(out=y_d[tcn * 128:(tcn + 1) * 128, :],
                                  in_=yv[:])

            if "2" in _PHASES:
                LOOK = _LOOK  # emit score pairs ahead so PE's in-order queue
                done_tc = 0   # never stalls on the exp/mask chain
                for gi, g in enumerate(GROUPS):
                    for gg in GROUPS[gi:gi + 1 + LOOK]:
                        for _c, ti, _to, _oo, _ln in gg["segs"]:
                            if ti not in emitted:
                                emit_pair(ti)
                    do_group(g)
                    # normalize runs one group behind so the bcast/mul never
                    # wait mid-queue on this group's recip
                    if gi > 0:
                        gp = GROUPS[gi - 1]
                        normalize(gp)
                        rows_done = gp["rows"][-1] + 1
                        while ("4" in _PHASES and done_tc < 32
                               and rows_done >= (done_tc + 1) * 2):
                            project(done_tc)
                            done_tc += 1
                normalize(GROUPS[-1])
                while "4" in _PHASES and done_tc < 32:
                    project(done_tc)
                    done_tc += 1
            opp_cm.__exit__(None, None, None)
    nc.compile()
    return nc


def _get_module():
    if "nc" not in _NC_CACHE:
        _NC_CACHE["nc"] = _build_module()
    return _NC_CACHE["nc"]


# ---------------------------------------------------------------- host

def kernel(x, w_qkv, w_out):
    x = np.asarray(x, np.float32)
    w_qkv = np.asarray(w_qkv, np.float32)
    w_out = np.asarray(w_out, np.float32)
    nc = _get_module()

    bf = ml_dtypes.bfloat16
    masks_pair = np.ascontiguousarray(
        np.broadcast_to(MASKS[:, :, None, :], (NMASK, 128, 2, 512))).astype(bf)
    xT = [np.ascontiguousarray(x[b].reshape(S, D).T).astype(bf) for b in range(B)]
    w_outT = np.ascontiguousarray(w_out.T)

    in_maps = []
    for c in range(NCORES):
        b, h0 = c // 4, 2 * (c % 4)
        f = h0 * 64
        in_maps.append({
            "xT": xT[b],
            "wq": np.ascontiguousarray(w_qkv[f:f + 128].T).astype(bf),
            "wk": np.ascontiguousarray(w_qkv[512 + f:512 + f + 128].T).astype(bf),
            "wv": np.ascontiguousarray(w_qkv[1024 + f:1024 + f + 128].T).astype(bf),
            "wo": np.ascontiguousarray(w_outT[f:f + 128]).astype(bf),
            "masks": masks_pair,
        })
    res = run_bass_kernel_spmd(nc, in_maps, list(range(NCORES)), trace=TRACE)
    global LAST_RESULTS
    LAST_RESULTS = res
    y = np.zeros((B, S, D), np.float32)
    for c in range(NCORES):
        y[c // 4] += res.results[c]["y"].astype(np.float32)
    return y.reshape(B, H, W, D)
